# revision 5
# baseline (speedup 1.0000x reference)
"""Trainium2 Bass kernel for nn_MultiHeadAttention (B=4, S=2048, H=16, D=64).

Sharding: 8 cores = 4 batches x 2 head-groups (8 heads each). Attention is
fully local per core; the output projection is column-sharded with a pairwise
AllGather of per-head attention outputs between the two cores of a batch.

Math folds (all exact):
- Q projection folded away: energy^T = kT^T (wk^T wq) qT, so only K is
  projected (with M = wk^T wq). The 1/sqrt(d_model)=1/32 scale is folded in.
- q/k biases: softmax(e + u[k] + w[q] + c) == softmax(e + u[k]) per column,
  so only u = kT^T (wk^T bq)/32 survives; it rides the ACT bias port.
- Softmax denominator: ones column appended to V via augmented projection;
  attn@V then yields [V^T P; 1^T P] so row 64 is the denominator.
- v bias: O_norm@wo + (bv tiled)@wo folds into bo_eff host-side.

Everything runs in fp32r (11-bit mantissa, full PE rate at N>=512).
"""

import numpy as np

import concourse.bass as bass
import concourse.mybir as mybir
import concourse.tile as tile
from concourse import bacc
from concourse.bass_utils import run_bass_kernel_spmd

f32 = mybir.dt.float32
f32r = mybir.dt.float32r

B, S, H, D = 4, 2048, 16, 64
HPC = 8  # heads per core
NB = 512  # matmul moving-dim chunk
QB = 1024  # q block for exp/attn@V
NKB = S // 128  # 16 k-blocks
EXP = mybir.ActivationFunctionType.Exp


def round_fp32r(x: np.ndarray) -> np.ndarray:
    b = np.ascontiguousarray(x.astype(np.float32)).view(np.uint32)
    return ((b + 0x800) & 0xFFFFF000).view(np.float32)


def build(reps=1):
    nc = bacc.Bacc("TRN2", target_bir_lowering=False, num_devices=8)

    kt = nc.dram_tensor("kt", [HPC, D, S], f32r, kind="ExternalInput")
    qt = nc.dram_tensor("qt", [HPC, D, S], f32r, kind="ExternalInput")
    vta = nc.dram_tensor("vta", [HPC, D + 1, S], f32r, kind="ExternalInput")
    m32 = nc.dram_tensor("m32", [D, D], f32r, kind="ExternalInput")
    wub = nc.dram_tensor("wub", [D, 2], f32r, kind="ExternalInput")
    wvta = nc.dram_tensor("wvta", [D + 1, D + 2], f32r, kind="ExternalInput")
    wot = nc.dram_tensor("wot", [HPC, 128, 512], f32r, kind="ExternalInput")
    boe = nc.dram_tensor("boe", [128, 4], f32, kind="ExternalInput")
    ones64 = nc.dram_tensor("ones64", [1, D], f32r, kind="ExternalInput")
    out = nc.dram_tensor("out", [512, S], f32, kind="ExternalOutput")

    with tile.TileContext(nc) as tc:
        with tc.tile_pool(name="dram", bufs=1, space="DRAM") as dram:
            for r in range(reps):
                ccin = [dram.tile([D, S], f32r, tag=f"ccin{r}_{p}", name=f"ccin{r}_{p}") for p in range(HPC)]
                ccout = [dram.tile([128, S], f32r, tag=f"ccout{r}_{p}", name=f"ccout{r}_{p}") for p in range(HPC)]
                _phase_ab(nc, tc, kt, qt, vta, m32, wub, wvta, ones64, ccin, ccout)
                _phase_c(nc, tc, wot, boe, out, ccout)
    nc.compile()
    return nc


def _phase_ab(nc, tc, kt, qt, vta, m32, wub, wvta, ones64, ccin, ccout):
    with tc.tile_pool(name="keep", bufs=1) as keep:
            m32_s = keep.tile([D, D], f32r, tag="m32")
            nc.default_dma_engine.dma_start(out=m32_s, in_=m32[:])
            wub_s = keep.tile([D, 2], f32r, tag="wub")
            nc.default_dma_engine.dma_start(out=wub_s, in_=wub[:])
            wvta_s = keep.tile([D + 1, D + 2], f32r, tag="wvta")
            nc.default_dma_engine.dma_start(out=wvta_s, in_=wvta[:])
            ones_s = keep.tile([1, D], f32r, tag="ones64")
            nc.default_dma_engine.dma_start(out=ones_s, in_=ones64[:])

            # Persistent per-pair tensors
            Qraw = [keep.tile([D, S], f32r, tag=f"Qraw{p}", name=f"Qraw{p}") for p in range(HPC)]
            Ks = [keep.tile([D, S], f32r, tag=f"Ks{p}", name=f"Ks{p}") for p in range(HPC)]
            Vs = [keep.tile([128, NKB, D + 2], f32r, tag=f"Vs{p}", name=f"Vs{p}") for p in range(HPC)]
            u_s = [keep.tile([128, 2 * NKB], f32, tag=f"u{p}", name=f"u{p}") for p in range(HPC)]

            # ---- Phase A: load raws + project K (with M32), u, V(+ones) ----
            with (
                tc.tile_pool(name="raw", bufs=2) as raw,
                tc.tile_pool(name="psk", bufs=1, space="PSUM") as pskp,
                tc.tile_pool(name="psu", bufs=2, space="PSUM") as psup,
                tc.tile_pool(name="psv", bufs=2, space="PSUM") as psvp,
            ):
                for p in range(HPC):
                    kt_t = raw.tile([D, S], f32r, tag="kt")
                    nc.default_dma_engine.dma_start(out=kt_t, in_=kt[p])
                    vta_t = raw.tile([D + 1, S], f32r, tag="vta")
                    nc.default_dma_engine.dma_start(out=vta_t, in_=vta[p])
                    nc.default_dma_engine.dma_start(out=Qraw[p], in_=qt[p])

                    # u[k] = kT^T wub, one column per k-block
                    psu = psup.tile([128, 2 * NKB], f32, tag="psu")
                    for kb in range(NKB):
                        nc.tensor.matmul(
                            psu[:, 2 * kb : 2 * kb + 2],
                            lhsT=kt_t[:, kb * 128 : (kb + 1) * 128],
                            rhs=wub_s[:],
                            start=True,
                            stop=True,
                        )
                    nc.vector.tensor_copy(u_s[p][:], psu[:])

                    # K~ = M32^T kT
                    psk = pskp.tile([D, S], f32, tag="psk")
                    for ch in range(S // NB):
                        nc.tensor.matmul(
                            psk[:, ch * NB : (ch + 1) * NB],
                            lhsT=m32_s[:],
                            rhs=kt_t[:, ch * NB : (ch + 1) * NB],
                            start=True,
                            stop=True,
                        )
                    nc.vector.tensor_copy(Ks[p][:], psk[:])

                    # V_aug = vta^T wvta  (ones column via augmented weight)
                    for grp in range(NKB // 4):
                        psv = psvp.tile([128, 4, D + 2], f32, tag="psv")
                        for j in range(4):
                            kb = grp * 4 + j
                            nc.tensor.matmul(
                                psv[:, j, :],
                                lhsT=vta_t[:, kb * 128 : (kb + 1) * 128],
                                rhs=wvta_s[:],
                                start=True,
                                stop=True,
                            )
                        nc.vector.tensor_copy(
                            Vs[p][:, grp * 4 : (grp + 1) * 4, :], psv[:]
                        )

            # ---- Phase B: attention per pair ----
            with (
                tc.tile_pool(name="scp", bufs=2, space="PSUM") as scp,
                tc.tile_pool(name="ovp", bufs=2, space="PSUM") as ovp,
                tc.tile_pool(name="pex", bufs=3) as pex,
                tc.tile_pool(name="onp", bufs=2) as onp,
                tc.tile_pool(name="nrm", bufs=2) as nrm,
            ):
                for p in range(HPC):
                    On = onp.tile([D, S], f32r, tag="On")
                    for qb in range(S // QB):
                        q0 = qb * QB
                        ov = ovp.tile([D + 1, QB], f32, tag="ov")
                        for kb in range(NKB):
                            sc = scp.tile([128, QB], f32, tag="sc")
                            for h in range(QB // NB):
                                nc.tensor.matmul(
                                    sc[:, h * NB : (h + 1) * NB],
                                    lhsT=Ks[p][:, kb * 128 : (kb + 1) * 128],
                                    rhs=Qraw[p][:, q0 + h * NB : q0 + (h + 1) * NB],
                                    start=True,
                                    stop=True,
                                )
                            pt = pex.tile([128, QB], f32r, tag="pt")
                            nc.scalar.activation(
                                pt[:], sc[:], EXP, bias=u_s[p][:, 2 * kb : 2 * kb + 1], scale=1.0
                            )
                            for h in range(QB // NB):
                                nc.tensor.matmul(
                                    ov[:, h * NB : (h + 1) * NB],
                                    lhsT=Vs[p][:, kb, 0 : D + 1],
                                    rhs=pt[:, h * NB : (h + 1) * NB],
                                    start=(kb == 0),
                                    stop=(kb == NKB - 1),
                                )
                        rcp = nrm.tile([1, QB], f32r, tag="rcp")
                        with nc.allow_low_precision(reason="fp32r softmax denom"):
                            nc.vector.reciprocal(rcp[:], ov[D : D + 1, :])
                        bc = scp.tile([128, QB], f32, tag="sc")
                        for h in range(QB // NB):
                            nc.tensor.matmul(
                                bc[0:D, h * NB : (h + 1) * NB],
                                lhsT=ones_s[:],
                                rhs=rcp[:, h * NB : (h + 1) * NB],
                                start=True,
                                stop=True,
                            )
                        oc = nrm.tile([D, QB], f32, tag="oc")
                        nc.vector.tensor_copy(oc[:], ov[0:D, :])
                        nc.vector.tensor_mul(On[:, q0 : q0 + QB], oc[:], bc[0:D, :])
                    nc.default_dma_engine.dma_start(out=ccin[p][:], in_=On[:])
                    nc.gpsimd.collective_compute(
                        "AllGather",
                        mybir.AluOpType.bypass,
                        replica_groups=[[0, 1], [2, 3], [4, 5], [6, 7]],
                        ins=[ccin[p].opt()],
                        outs=[ccout[p].opt()],
                    )


def _phase_c(nc, tc, wot, boe, out, ccout):
    if True:
            # ---- Phase C: column-sharded output projection ----
            with (
                tc.tile_pool(name="pcw", bufs=1) as pcw,
                tc.tile_pool(name="pco", bufs=1) as pco,
                tc.tile_pool(name="fin", bufs=2) as finp,
                tc.tile_pool(name="fps", bufs=2, space="PSUM") as fpsp,
            ):
                boe_s = pcw.tile([128, 4], f32, tag="boe")
                nc.default_dma_engine.dma_start(out=boe_s, in_=boe[:])
                wo_s = []
                Ob = []
                for t in range(HPC):
                    w = pcw.tile([128, 512], f32r, tag=f"wo{t}", name=f"wo{t}")
                    nc.default_dma_engine.dma_start(out=w, in_=wot[t])
                    wo_s.append(w)
                    o = pco.tile([128, S], f32r, tag=f"Ob{t}", name=f"Ob{t}")
                    nc.default_dma_engine.dma_start(out=o, in_=ccout[t][:])
                    Ob.append(o)
                for ob in range(4):
                    for qc in range(S // NB):
                        fp_ = fpsp.tile([128, NB], f32, tag="fp")
                        for t in range(HPC):
                            nc.tensor.matmul(
                                fp_[:],
                                lhsT=wo_s[t][:, ob * 128 : (ob + 1) * 128],
                                rhs=Ob[t][:, qc * NB : (qc + 1) * NB],
                                start=(t == 0),
                                stop=(t == HPC - 1),
                            )
                        fo = finp.tile([128, NB], f32, tag="fo")
                        nc.vector.tensor_scalar_add(fo[:], fp_[:], boe_s[:, ob : ob + 1])
                        nc.default_dma_engine.dma_start(
                            out=out[ob * 128 : (ob + 1) * 128, qc * NB : (qc + 1) * NB],
                            in_=fo[:],
                        )


_NC_CACHE = {}


def _get_nc(reps=1):
    if reps not in _NC_CACHE:
        _NC_CACHE[reps] = build(reps)
    return _NC_CACHE[reps]


def _prep_core_inputs(values, keys, query, wq, bq, wk, bk, wv, bv, wo, bo):
    """Build the 8 per-core input maps (host-side shard + layout prep)."""
    del bk  # cancels in softmax (q-only term)
    m32 = round_fp32r((wk.T.astype(np.float64) @ wq.astype(np.float64)) / 32.0)
    wub1 = (wk.T.astype(np.float64) @ bq.astype(np.float64)) / 32.0
    wub = round_fp32r(
        np.concatenate([wub1.reshape(D, 1), np.zeros((D, 1))], axis=1)
    )
    wvta = np.zeros((D + 1, D + 2), np.float32)
    wvta[:D, :D] = wv.T
    wvta[D, D] = 1.0
    wvta = round_fp32r(wvta)
    ones64 = np.ones((1, D), np.float32)

    bv_full = np.tile(bv, H)
    bo_eff = (
        bo.astype(np.float64) + wo.astype(np.float64) @ bv_full.astype(np.float64)
    ).astype(np.float32)
    woT = wo.T  # [in 1024, out 1024]

    in_maps = []
    for c in range(8):
        b, g = c // 2, c % 2
        heads = slice(g * HPC, (g + 1) * HPC)
        # [S, H, D] -> [h, D, S]
        kt = round_fp32r(
            keys[b].reshape(S, H, D)[:, heads, :].transpose(1, 2, 0)
        )
        qt = round_fp32r(
            query[b].reshape(S, H, D)[:, heads, :].transpose(1, 2, 0)
        )
        vt = round_fp32r(
            values[b].reshape(S, H, D)[:, heads, :].transpose(1, 2, 0)
        )
        vta = np.concatenate([vt, np.ones((HPC, 1, S), np.float32)], axis=1)
        # wo^T rows permuted to AllGather order: pair p = heads (p, p+8)
        ocols = slice(g * 512, (g + 1) * 512)
        wot = np.stack(
            [
                np.concatenate(
                    [
                        woT[p * D : (p + 1) * D, ocols],
                        woT[(p + HPC) * D : (p + HPC + 1) * D, ocols],
                    ]
                )
                for p in range(HPC)
            ]
        )
        boe = np.ascontiguousarray(
            bo_eff[g * 512 : (g + 1) * 512].reshape(4, 128).T
        )
        in_maps.append(
            dict(
                kt=np.ascontiguousarray(kt),
                qt=np.ascontiguousarray(qt),
                vta=np.ascontiguousarray(vta),
                m32=m32,
                wub=wub,
                wvta=wvta,
                wot=round_fp32r(wot),
                boe=boe,
                ones64=ones64,
            )
        )
    return in_maps


def kernel(values, keys, query, wq, bq, wk, bk, wv, bv, wo, bo):
    values = np.asarray(values, np.float32)
    keys = np.asarray(keys, np.float32)
    query = np.asarray(query, np.float32)
    in_maps = _prep_core_inputs(
        values, keys, query,
        np.asarray(wq, np.float32), np.asarray(bq, np.float32),
        np.asarray(wk, np.float32), np.asarray(bk, np.float32),
        np.asarray(wv, np.float32), np.asarray(bv, np.float32),
        np.asarray(wo, np.float32), np.asarray(bo, np.float32),
    )
    nc = _get_nc()
    res = run_bass_kernel_spmd(nc, in_maps, list(range(8)))
    out = np.empty((B, S, 1024), np.float32)
    for c in range(8):
        b, g = c // 2, c % 2
        out[b, :, g * 512 : (g + 1) * 512] = res.results[c]["out"].T
    return out


# revision 6
# speedup vs baseline: 1.6848x; 1.6848x over previous
"""Trainium2 Bass kernel for nn_MultiHeadAttention (B=4, S=2048, H=16, D=64).

Sharding: 8 cores = 4 batches x 2 head-groups (8 heads each). Attention is
fully local per core; the output projection is column-sharded with a pairwise
AllGather of per-head attention outputs between the two cores of a batch.

Math folds (all exact):
- Q projection folded away: energy^T = kT^T (wk^T wq) qT, so only K is
  projected (with M = wk^T wq). The 1/sqrt(d_model)=1/32 scale is folded in.
- q/k biases: softmax(e + u[k] + w[q] + c) == softmax(e + u[k]) per column,
  so only u = kT^T (wk^T bq)/32 survives; it rides the ACT bias port.
- Softmax denominator: ones column appended to V via augmented projection;
  attn@V then yields [V^T P; 1^T P] so row 64 is the denominator.
- v bias: O_norm@wo + (bv tiled)@wo folds into bo_eff host-side.

Everything runs in fp32r (11-bit mantissa, full PE rate at N>=512).
"""

import numpy as np

import concourse.bass as bass
import concourse.mybir as mybir
import concourse.tile as tile
from concourse import bacc
from concourse.bass_utils import run_bass_kernel_spmd

f32 = mybir.dt.float32
f32r = mybir.dt.float32r

B, S, H, D = 4, 2048, 16, 64
HPC = 8  # heads per core
NB = 512  # matmul moving-dim chunk
QB = 1024  # q block for exp/attn@V
NKB = S // 128  # 16 k-blocks
EXP = mybir.ActivationFunctionType.Exp


def round_fp32r(x: np.ndarray) -> np.ndarray:
    b = np.ascontiguousarray(x.astype(np.float32)).view(np.uint32)
    return ((b + 0x800) & 0xFFFFF000).view(np.float32)


def build(reps=1, use_cc=True):
    nc = bacc.Bacc("TRN2", target_bir_lowering=False, num_devices=8)

    kt = nc.dram_tensor("kt", [HPC, D, S], f32r, kind="ExternalInput")
    qt = nc.dram_tensor("qt", [HPC, D, S], f32r, kind="ExternalInput")
    vta = nc.dram_tensor("vta", [HPC, D + 1, S], f32r, kind="ExternalInput")
    m32 = nc.dram_tensor("m32", [D, D], f32r, kind="ExternalInput")
    wub = nc.dram_tensor("wub", [D, 2], f32r, kind="ExternalInput")
    wvta = nc.dram_tensor("wvta", [D + 1, D + 2], f32r, kind="ExternalInput")
    wot = nc.dram_tensor("wot", [HPC, 128, 512], f32r, kind="ExternalInput")
    boe = nc.dram_tensor("boe", [128, 4], f32, kind="ExternalInput")
    ones64 = nc.dram_tensor("ones64", [1, D], f32r, kind="ExternalInput")
    out = nc.dram_tensor("out", [512, S], f32, kind="ExternalOutput")

    with tile.TileContext(nc) as tc:
        with tc.tile_pool(name="dram", bufs=1, space="DRAM") as dram:
            for r in range(reps):
                ccin = [dram.tile([D, S], f32r, tag=f"ccin{r}_{p}", name=f"ccin{r}_{p}") for p in range(HPC)]
                ccout = [dram.tile([128, S], f32r, tag=f"ccout{r}_{p}", name=f"ccout{r}_{p}") for p in range(HPC)]
                _phase_ab(nc, tc, kt, qt, vta, m32, wub, wvta, ones64, ccin, ccout, use_cc)
                _phase_c(nc, tc, wot, boe, out, ccout)
    nc.compile()
    return nc


def _phase_ab(nc, tc, kt, qt, vta, m32, wub, wvta, ones64, ccin, ccout, use_cc):
    with tc.tile_pool(name="keep", bufs=1) as keep:
            m32_s = keep.tile([D, D], f32r, tag="m32")
            nc.default_dma_engine.dma_start(out=m32_s, in_=m32[:])
            wub_s = keep.tile([D, 2], f32r, tag="wub")
            nc.default_dma_engine.dma_start(out=wub_s, in_=wub[:])
            wvta_s = keep.tile([D + 1, D + 2], f32r, tag="wvta")
            nc.default_dma_engine.dma_start(out=wvta_s, in_=wvta[:])
            ones_s = keep.tile([1, D], f32r, tag="ones64")
            nc.default_dma_engine.dma_start(out=ones_s, in_=ones64[:])

            # Persistent per-pair tensors
            Qraw = [keep.tile([D, S], f32r, tag=f"Qraw{p}", name=f"Qraw{p}") for p in range(HPC)]
            Ks = [keep.tile([D, S], f32r, tag=f"Ks{p}", name=f"Ks{p}") for p in range(HPC)]
            Vs = [keep.tile([128, NKB, D + 2], f32r, tag=f"Vs{p}", name=f"Vs{p}") for p in range(HPC)]
            u_s = [keep.tile([128, 2 * NKB], f32, tag=f"u{p}", name=f"u{p}") for p in range(HPC)]

            # ---- Phase A: load raws + project K (with M32), u, V(+ones) ----
            with (
                tc.tile_pool(name="raw", bufs=2) as raw,
                tc.tile_pool(name="psk", bufs=1, space="PSUM") as pskp,
                tc.tile_pool(name="psu", bufs=2, space="PSUM") as psup,
                tc.tile_pool(name="psv", bufs=2, space="PSUM") as psvp,
            ):
                for p in range(HPC):
                    kt_t = raw.tile([D, S], f32r, tag="kt")
                    nc.default_dma_engine.dma_start(out=kt_t, in_=kt[p])
                    vta_t = raw.tile([D + 1, S], f32r, tag="vta")
                    nc.default_dma_engine.dma_start(out=vta_t, in_=vta[p])
                    nc.default_dma_engine.dma_start(out=Qraw[p], in_=qt[p])

                    # u[k] = kT^T wub, one column per k-block
                    psu = psup.tile([128, 2 * NKB], f32, tag="psu")
                    for kb in range(NKB):
                        nc.tensor.matmul(
                            psu[:, 2 * kb : 2 * kb + 2],
                            lhsT=kt_t[:, kb * 128 : (kb + 1) * 128],
                            rhs=wub_s[:],
                            start=True,
                            stop=True,
                        )
                    nc.vector.tensor_copy(u_s[p][:], psu[:])

                    # K~ = M32^T kT
                    psk = pskp.tile([D, S], f32, tag="psk")
                    for ch in range(S // NB):
                        nc.tensor.matmul(
                            psk[:, ch * NB : (ch + 1) * NB],
                            lhsT=m32_s[:],
                            rhs=kt_t[:, ch * NB : (ch + 1) * NB],
                            start=True,
                            stop=True,
                        )
                    nc.vector.tensor_copy(Ks[p][:], psk[:])

                    # V_aug = vta^T wvta  (ones column via augmented weight)
                    for grp in range(NKB // 4):
                        psv = psvp.tile([128, 4, D + 2], f32, tag="psv")
                        for j in range(4):
                            kb = grp * 4 + j
                            nc.tensor.matmul(
                                psv[:, j, :],
                                lhsT=vta_t[:, kb * 128 : (kb + 1) * 128],
                                rhs=wvta_s[:],
                                start=True,
                                stop=True,
                            )
                        nc.vector.tensor_copy(
                            Vs[p][:, grp * 4 : (grp + 1) * 4, :], psv[:]
                        )

            # ---- Phase B: attention per pair ----
            with (
                tc.tile_pool(name="scp", bufs=2, space="PSUM") as scp,
                tc.tile_pool(name="ovp", bufs=2, space="PSUM") as ovp,
                tc.tile_pool(name="pex", bufs=3) as pex,
                tc.tile_pool(name="onp", bufs=2) as onp,
                tc.tile_pool(name="nrm", bufs=2) as nrm,
            ):
                for p in range(HPC):
                    On = onp.tile([D, S], f32r, tag="On")
                    for qb in range(S // QB):
                        q0 = qb * QB
                        ov = ovp.tile([D + 1, QB], f32, tag="ov")
                        for kb in range(NKB):
                            sc = scp.tile([128, QB], f32, tag="sc")
                            for h in range(QB // NB):
                                nc.tensor.matmul(
                                    sc[:, h * NB : (h + 1) * NB],
                                    lhsT=Ks[p][:, kb * 128 : (kb + 1) * 128],
                                    rhs=Qraw[p][:, q0 + h * NB : q0 + (h + 1) * NB],
                                    start=True,
                                    stop=True,
                                )
                            pt = pex.tile([128, QB], f32r, tag="pt")
                            nc.scalar.activation(
                                pt[:], sc[:], EXP, bias=u_s[p][:, 2 * kb : 2 * kb + 1], scale=1.0
                            )
                            for h in range(QB // NB):
                                nc.tensor.matmul(
                                    ov[:, h * NB : (h + 1) * NB],
                                    lhsT=Vs[p][:, kb, 0 : D + 1],
                                    rhs=pt[:, h * NB : (h + 1) * NB],
                                    start=(kb == 0),
                                    stop=(kb == NKB - 1),
                                )
                        rcp = nrm.tile([1, QB], f32r, tag="rcp")
                        with nc.allow_low_precision(reason="fp32r softmax denom"):
                            nc.vector.reciprocal(rcp[:], ov[D : D + 1, :])
                        bc = scp.tile([128, QB], f32, tag="sc")
                        for h in range(QB // NB):
                            nc.tensor.matmul(
                                bc[0:D, h * NB : (h + 1) * NB],
                                lhsT=ones_s[:],
                                rhs=rcp[:, h * NB : (h + 1) * NB],
                                start=True,
                                stop=True,
                            )
                        oc = nrm.tile([D, QB], f32, tag="oc")
                        nc.vector.tensor_copy(oc[:], ov[0:D, :])
                        nc.vector.tensor_mul(On[:, q0 : q0 + QB], oc[:], bc[0:D, :])
                    nc.default_dma_engine.dma_start(out=ccin[p][:], in_=On[:])
                    if not use_cc:
                        continue
                    nc.gpsimd.collective_compute(
                        "AllGather",
                        mybir.AluOpType.bypass,
                        replica_groups=[[0, 1], [2, 3], [4, 5], [6, 7]],
                        ins=[ccin[p].opt()],
                        outs=[ccout[p].opt()],
                    )


def _phase_c(nc, tc, wot, boe, out, ccout):
    if True:
            # ---- Phase C: column-sharded output projection ----
            with (
                tc.tile_pool(name="pcw", bufs=1) as pcw,
                tc.tile_pool(name="pco", bufs=1) as pco,
                tc.tile_pool(name="fin", bufs=2) as finp,
                tc.tile_pool(name="fps", bufs=2, space="PSUM") as fpsp,
            ):
                boe_s = pcw.tile([128, 4], f32, tag="boe")
                nc.default_dma_engine.dma_start(out=boe_s, in_=boe[:])
                wo_s = []
                Ob = []
                for t in range(HPC):
                    w = pcw.tile([128, 512], f32r, tag=f"wo{t}", name=f"wo{t}")
                    nc.default_dma_engine.dma_start(out=w, in_=wot[t])
                    wo_s.append(w)
                    o = pco.tile([128, S], f32r, tag=f"Ob{t}", name=f"Ob{t}")
                    nc.default_dma_engine.dma_start(out=o, in_=ccout[t][:])
                    Ob.append(o)
                for ob in range(4):
                    for qc in range(S // NB):
                        fp_ = fpsp.tile([128, NB], f32, tag="fp")
                        for t in range(HPC):
                            nc.tensor.matmul(
                                fp_[:],
                                lhsT=wo_s[t][:, ob * 128 : (ob + 1) * 128],
                                rhs=Ob[t][:, qc * NB : (qc + 1) * NB],
                                start=(t == 0),
                                stop=(t == HPC - 1),
                            )
                        fo = finp.tile([128, NB], f32, tag="fo")
                        nc.vector.tensor_scalar_add(fo[:], fp_[:], boe_s[:, ob : ob + 1])
                        nc.default_dma_engine.dma_start(
                            out=out[ob * 128 : (ob + 1) * 128, qc * NB : (qc + 1) * NB],
                            in_=fo[:],
                        )


_NC_CACHE = {}


def _get_nc(reps=1, use_cc=True):
    key = (reps, use_cc)
    if key not in _NC_CACHE:
        _NC_CACHE[key] = build(reps, use_cc)
    return _NC_CACHE[key]


def _prep_core_inputs(values, keys, query, wq, bq, wk, bk, wv, bv, wo, bo):
    """Build the 8 per-core input maps (host-side shard + layout prep)."""
    del bk  # cancels in softmax (q-only term)
    m32 = round_fp32r((wk.T.astype(np.float64) @ wq.astype(np.float64)) / 32.0)
    wub1 = (wk.T.astype(np.float64) @ bq.astype(np.float64)) / 32.0
    wub = round_fp32r(
        np.concatenate([wub1.reshape(D, 1), np.zeros((D, 1))], axis=1)
    )
    wvta = np.zeros((D + 1, D + 2), np.float32)
    wvta[:D, :D] = wv.T
    wvta[D, D] = 1.0
    wvta = round_fp32r(wvta)
    ones64 = np.ones((1, D), np.float32)

    bv_full = np.tile(bv, H)
    bo_eff = (
        bo.astype(np.float64) + wo.astype(np.float64) @ bv_full.astype(np.float64)
    ).astype(np.float32)
    woT = wo.T  # [in 1024, out 1024]

    in_maps = []
    for c in range(8):
        b, g = c // 2, c % 2
        heads = slice(g * HPC, (g + 1) * HPC)
        # [S, H, D] -> [h, D, S]
        kt = round_fp32r(
            keys[b].reshape(S, H, D)[:, heads, :].transpose(1, 2, 0)
        )
        qt = round_fp32r(
            query[b].reshape(S, H, D)[:, heads, :].transpose(1, 2, 0)
        )
        vt = round_fp32r(
            values[b].reshape(S, H, D)[:, heads, :].transpose(1, 2, 0)
        )
        vta = np.concatenate([vt, np.ones((HPC, 1, S), np.float32)], axis=1)
        # wo^T rows permuted to AllGather order: pair p = heads (p, p+8)
        ocols = slice(g * 512, (g + 1) * 512)
        wot = np.stack(
            [
                np.concatenate(
                    [
                        woT[p * D : (p + 1) * D, ocols],
                        woT[(p + HPC) * D : (p + HPC + 1) * D, ocols],
                    ]
                )
                for p in range(HPC)
            ]
        )
        boe = np.ascontiguousarray(
            bo_eff[g * 512 : (g + 1) * 512].reshape(4, 128).T
        )
        in_maps.append(
            dict(
                kt=np.ascontiguousarray(kt),
                qt=np.ascontiguousarray(qt),
                vta=np.ascontiguousarray(vta),
                m32=m32,
                wub=wub,
                wvta=wvta,
                wot=round_fp32r(wot),
                boe=boe,
                ones64=ones64,
            )
        )
    return in_maps


def kernel(values, keys, query, wq, bq, wk, bk, wv, bv, wo, bo):
    values = np.asarray(values, np.float32)
    keys = np.asarray(keys, np.float32)
    query = np.asarray(query, np.float32)
    in_maps = _prep_core_inputs(
        values, keys, query,
        np.asarray(wq, np.float32), np.asarray(bq, np.float32),
        np.asarray(wk, np.float32), np.asarray(bk, np.float32),
        np.asarray(wv, np.float32), np.asarray(bv, np.float32),
        np.asarray(wo, np.float32), np.asarray(bo, np.float32),
    )
    nc = _get_nc()
    res = run_bass_kernel_spmd(nc, in_maps, list(range(8)))
    out = np.empty((B, S, 1024), np.float32)
    for c in range(8):
        b, g = c // 2, c % 2
        out[b, :, g * 512 : (g + 1) * 512] = res.results[c]["out"].T
    return out
